# revision 23
# baseline (speedup 1.0000x reference)
"""Causal multi-head self-attention with RoPE on 8 Trainium2 NeuronCores.

Full-input contract: kernel(**inputs) takes the complete tensors and returns
the complete [B, S, D] output. Sharding: core c handles batch c//2 and heads
(c%2)*8 .. (c%2)*8+8 (2-way tensor parallel within each batch pair).

Per core:
  - q/k/v projections (f32r matmuls, fp32 PSUM) + RoPE on DVE. Rotated q/k are
    written with stride-2 partition APs into per-head-pair tiles whose rows
    interleave the rotary E/O dims, so score matmuls contract K=64 in ONE op.
  - attention: per head-pair, per sk-tile: 2 score matmuls -> one 2-head-wide
    exp (ACT) from a 2-bank PSUM tile -> causal mask of only the 128-wide
    diagonal block (Pool) -> 2 PV matmuls. Diagonal sk-tiles are narrowed to
    the live query range. v carries a ones column so PV row 64 accumulates the
    softmax denominator for free.
  - the per-head ot ([64, S] normalized attention output, bf16) is exchanged
    between the pair cores with one AllGather per half (overlaps compute);
    each core then computes HALF the output features of the final projection
    (bf16 matmuls) over all 16 heads, so no AllReduce is needed.

Host side reassembles out[b] = concat(core 2b rows, core 2b+1 rows).T.
"""

import numpy as np
import ml_dtypes

import concourse.bass as bass
import concourse.mybir as mybir
import concourse.tile as tile
from concourse import bacc
from concourse.bass_utils import run_bass_kernel_spmd

F32 = mybir.dt.float32
F32R = mybir.dt.float32r
BF16 = mybir.dt.bfloat16
AF = mybir.ActivationFunctionType
ALU = mybir.AluOpType

P = 128          # partitions
SQ = 512         # moving-dim chunk (max for 4-byte dtypes)
DK = 64          # head dim
NH = 8           # heads per core
DLOC = NH * DK   # 512 local out-features for q/k/v
THETA = 10000.0

B, S, D, H = 4, 2048, 1024, 16
N_CORES = 8


PHASE_MARKS = []


def _mark(nc, label):
    PHASE_MARKS.append((nc.next_id(), label))


def build_attention_program(DIN=D, DOUT=D, SEQ=S, all_reduce=True, groups=None, reps=1):
    """One SPMD Bass program. Per-core DRAM I/O:
      xt   [DIN, SEQ]  f32   x[b].T
      wqt  [DIN, DLOC] f32   wq rows (perm: per-half [E-block|O-block]) transposed
      wkt  [DIN, DLOC] f32   likewise
      wvt  [DIN, DLOC] f32   wv rows (perm2: per-head [even|odd]) transposed
      wot  [DIN, DOUT//2] bf16  wo[out-slice, gathered-head-order].T
      cos4 [P, SEQ] f32      cos table, 4x stacked [32, SEQ]
      sin4 [P, SEQ] f32
      outp [DOUT//2, SEQ] f32   this core's slice of out[b].T
    """
    KC = DIN // P        # contraction chunks (8)
    SJ = SEQ // SQ       # sq chunks (4)
    STJ = SQ // P        # 128-s-tiles per sq chunk (4)
    ST = SEQ // P        # total s-tiles (16)
    DO2 = DOUT // 2      # out-features computed by this core (512)
    OC = DO2 // P        # out-proj dout chunks (4)
    assert DIN % P == 0 and SEQ % SQ == 0

    nc = bacc.Bacc(
        "TRN2",
        target_bir_lowering=False,
        debug=False,
        num_devices=(len(groups) * len(groups[0]) if groups else 1),
    )
    xt = nc.declare_dram_parameter("xt", [DIN, SEQ], F32R, isOutput=False)
    wqt = nc.declare_dram_parameter("wqt", [DIN, DLOC], F32R, isOutput=False)
    wkt = nc.declare_dram_parameter("wkt", [DIN, DLOC], F32R, isOutput=False)
    wvt = nc.declare_dram_parameter("wvt", [DIN, DLOC], F32R, isOutput=False)
    wot = nc.declare_dram_parameter("wot", [DIN, DO2], BF16, isOutput=False)
    cos4 = nc.declare_dram_parameter("cos4", [P, SEQ], F32, isOutput=False)
    sin4 = nc.declare_dram_parameter("sin4", [P, SEQ], F32, isOutput=False)
    outp = nc.declare_dram_parameter("outp", [DO2, SEQ], F32, isOutput=True)

    from contextlib import ExitStack

    with tile.TileContext(nc) as tc, ExitStack() as ctx:
        ctx.enter_context(nc.allow_low_precision(reason="f32r carries full fp32 bytes"))
        consts = ctx.enter_context(tc.tile_pool(name="consts", bufs=1))
        tabs = ctx.enter_context(tc.tile_pool(name="tabs", bufs=2))
        wload = ctx.enter_context(tc.tile_pool(name="wload", bufs=2))
        xload = ctx.enter_context(tc.tile_pool(name="xload", bufs=2))
        qf_pool = ctx.enter_context(tc.tile_pool(name="qf", bufs=2))
        kf_pool = ctx.enter_context(tc.tile_pool(name="kf", bufs=1))
        v_pool = ctx.enter_context(tc.tile_pool(name="vp", bufs=1))
        ot_pool = ctx.enter_context(tc.tile_pool(name="ot", bufs=1))
        tmp_pool = ctx.enter_context(tc.tile_pool(name="tmp", bufs=1))
        pt_pool = ctx.enter_context(tc.tile_pool(name="pt", bufs=2))
        den_pool = ctx.enter_context(tc.tile_pool(name="den", bufs=1))
        po_pool = ctx.enter_context(tc.tile_pool(name="po", bufs=1))
        ob_pool = ctx.enter_context(tc.tile_pool(name="ob", bufs=2))
        dram_pool = ctx.enter_context(tc.tile_pool(name="dram", bufs=1, space="DRAM"))
        psA = ctx.enter_context(tc.tile_pool(name="psA", bufs=1, space="PSUM"))
        psS = ctx.enter_context(tc.tile_pool(name="psS", bufs=2, space="PSUM"))
        psV = ctx.enter_context(tc.tile_pool(name="psV", bufs=1, space="PSUM"))

        for _rep in range(reps):
            po_tiles = {}  # half -> [128, 4, SEQ] bf16 gathered (rank, chunk)
            ag_outs = {}   # (half, j) -> [2, 2, P, SQ] bf16 gathered chunk

            # preload both halves' qkv weights up front, spread across the
            # three DMA-issue queues so they overlap each other and the xt
            # loads (sync carries xt/cos/sin)
            w_sbs = {}
            for half in range(2):
                c0 = half * 256
                wq_sb = wload.tile([P, KC, 256], F32R, tag="wq", name=f"wq{half}")
                nc.gpsimd.dma_start(
                    wq_sb[:], wqt[:, c0 : c0 + 256].rearrange("(k p) c -> p k c", p=P)
                )
                wk_sb = wload.tile([P, KC, 256], F32R, tag="wk", name=f"wk{half}")
                nc.scalar.dma_start(
                    wk_sb[:], wkt[:, c0 : c0 + 256].rearrange("(k p) c -> p k c", p=P)
                )
                wv_sb = wload.tile([P, KC, 256], F32R, tag="wv", name=f"wv{half}")
                nc.gpsimd.dma_start(
                    wv_sb[:], wvt[:, c0 : c0 + 256].rearrange("(k p) c -> p k c", p=P)
                )
                w_sbs[half] = (wq_sb, wk_sb, wv_sb)
            # out-proj weights early too, but behind the qkv weights
            wo_sb = consts.tile([P, KC, DO2], BF16, tag="wo")
            nc.scalar.dma_start(wo_sb[:], wot.rearrange("(k p) c -> p k c", p=P))

            for half in range(2):  # heads 4*half .. 4*half+4 (local)
                wq_sb, wk_sb, wv_sb = w_sbs[half]

                # rotated k, rows per pair-tile: head (2p): dims interleaved
                # [E0,O0,E1,O1,...] rows 0:64; head (2p+1): rows 64:128
                kf = [
                    kf_pool.tile([P, SEQ], F32R, tag=f"kf{p}", name=f"kf{p}")
                    for p in range(2)
                ]
                # v natural [s, dv]: per s-tile, per head: 64 dims + ones col
                if half == 0:
                    ones_f32 = consts.tile([P, 1], F32, tag="one1")
                    nc.vector.memset(ones_f32[:], 1.0)
                v_sb = v_pool.tile([P, ST, 4, DK + 1], F32R, tag="v")
                nc.vector.tensor_copy(
                    v_sb[:, :, :, DK : DK + 1],
                    ones_f32[:, None, None, :].broadcast_to((P, ST, 4, 1)),
                )
                # normalized attention output, rows like kf pair-tiles but in
                # perm2 (per-head [even|odd]) order to match wvt/wot
                ot_sb = [
                    ot_pool.tile([P, SEQ], BF16, tag=f"ot{p}", name=f"ot{p}")
                    for p in range(2)
                ]

                for j in range(SJ):
                    _mark(nc, f"qkv h{half} j{j}")
                    js = slice(j * SQ, (j + 1) * SQ)
                    xt_sb = xload.tile([P, KC, SQ], F32R, tag="xt")
                    nc.sync.dma_start(
                        xt_sb[:], xt[:, js].rearrange("(k p) s -> p k s", p=P)
                    )
                    cos_j = tabs.tile([P, SQ], F32, tag="cosj")
                    nc.sync.dma_start(cos_j[:], cos4[:, js])
                    sin_j = tabs.tile([P, SQ], F32, tag="sinj")
                    nc.sync.dma_start(sin_j[:], sin4[:, js])

                    qf = [
                        qf_pool.tile([P, SQ], F32R, tag=f"qf{p}", name=f"qf{p}")
                        for p in range(2)
                    ]

                    # ---------------- q/k projections + RoPE ----------------
                    # psum tags alternate E/O so WAR serialization overlaps
                    for tname, wsb in (("q", wq_sb), ("k", wk_sb)):
                        qkps = {}
                        for eo in range(2):  # 0=E cols, 1=O cols
                            ps = psA.tile([P, SQ], F32, tag=("mmE", "mmO")[eo])
                            cc = eo * P
                            for kk in range(KC):
                                nc.tensor.matmul(
                                    ps[:],
                                    lhsT=(wsb[:, kk, cc : cc + P]),
                                    rhs=(xt_sb[:, kk, :]),
                                    start=(kk == 0),
                                    stop=(kk == KC - 1),
                                )
                            qkps[eo] = ps
                        psE, psO = qkps[0], qkps[1]
                        # RoPE: yE = cos*E - sin*O ; yO = sin*E + cos*O
                        # t1/t3 first so psE frees before psO's consumers run
                        t1 = tmp_pool.tile([P, SQ], F32, tag="t1")
                        nc.vector.tensor_tensor(t1[:], cos_j[:], psE[:], ALU.mult)
                        t3 = tmp_pool.tile([P, SQ], F32, tag="t3")
                        nc.vector.tensor_tensor(t3[:], sin_j[:], psE[:], ALU.mult)
                        t2 = tmp_pool.tile([P, SQ], F32, tag="t2")
                        nc.vector.tensor_tensor(t2[:], sin_j[:], psO[:], ALU.mult)
                        t4 = tmp_pool.tile([P, SQ], F32, tag="t4")
                        nc.vector.tensor_tensor(t4[:], cos_j[:], psO[:], ALU.mult)
                        # per-head contiguous 32-row writes: head h of pair p
                        # occupies rows [hh*64, hh*64+64) as [E(32); O(32)]
                        for h in range(4):
                            p, hh = h // 2, h % 2
                            src = slice(h * 32, (h + 1) * 32)
                            dst = qf[p] if tname == "q" else kf[p]
                            cols = slice(None) if tname == "q" else js
                            nc.vector.tensor_tensor(
                                dst[hh * DK : hh * DK + 32, cols],
                                t1[src, :],
                                t2[src, :],
                                ALU.subtract,
                            )
                            nc.vector.tensor_tensor(
                                dst[hh * DK + 32 : hh * DK + DK, cols],
                                t3[src, :],
                                t4[src, :],
                                ALU.add,
                            )

                    # ---------------- v projection ----------------
                    for st in range(STJ):
                        ps = psA.tile([P, 256], F32, tag=("mmE", "mmO")[st % 2])
                        for kk in range(KC):
                            nc.tensor.matmul(
                                ps[:],
                                lhsT=(xt_sb[:, kk, st * P : (st + 1) * P]),
                                rhs=(wv_sb[:, kk, :]),
                                start=(kk == 0),
                                stop=(kk == KC - 1),
                            )
                        nc.scalar.copy(
                            v_sb[:, j * STJ + st, :, 0:DK],
                            ps.rearrange("p (h d) -> p h d", h=4),
                        )

                    # ---------------- attention for this j ----------------
                    ntile = (j + 1) * STJ  # causal: sk-tiles 0..ntile-1
                    _mark(nc, f"attn h{half} j{j}")
                    for p in range(2):  # head pairs within the half
                        opv = psV.tile([DK + 1, 2, SQ], F32, tag="opv")
                        pending = None  # software pipeline: PV lags one tile
                        for t in range(ntile):
                            s0 = max(0, (t - STJ * j) * P)
                            ts_ = slice(t * P, (t + 1) * P)
                            ssc = psS.tile([P, 2, SQ], F32, tag="sc")
                            for hh in range(2):
                                hr = slice(hh * DK, (hh + 1) * DK)
                                nc.tensor.matmul(
                                    ssc[:, hh, s0:],
                                    lhsT=(kf[p][hr, ts_]),
                                    rhs=(qf[p][hr, s0:]),
                                    start=True,
                                    stop=True,
                                )
                            pt = pt_pool.tile([P, 2, SQ], F32R, tag="pt")
                            nc.scalar.activation(
                                pt[:, :, s0:], ssc[:, :, s0:], AF.Exp, scale=0.125
                            )
                            if t >= ntile - STJ:
                                # diagonal block: keep sq >= sk within the
                                # 128-wide block at cols s0:s0+128
                                nc.gpsimd.affine_select(
                                    out=pt[:, :, s0 : s0 + P],
                                    in_=pt[:, :, s0 : s0 + P],
                                    compare_op=ALU.is_ge,
                                    fill=0.0,
                                    base=0,
                                    pattern=[[0, 2], [1, P]],
                                    channel_multiplier=-1,
                                )
                            if pending is not None:
                                self_t, self_s0, self_pt = pending
                                for hh in range(2):
                                    nc.tensor.matmul(
                                        opv[:, hh, self_s0:],
                                        lhsT=(v_sb[:, self_t, p * 2 + hh, :]),
                                        rhs=(self_pt[:, hh, self_s0:]),
                                        start=(self_t == 0),
                                        stop=(self_t == ntile - 1),
                                    )
                            pending = (t, s0, pt)
                        self_t, self_s0, self_pt = pending
                        for hh in range(2):
                            nc.tensor.matmul(
                                opv[:, hh, self_s0:],
                                lhsT=(v_sb[:, self_t, p * 2 + hh, :]),
                                rhs=(self_pt[:, hh, self_s0:]),
                                start=(self_t == 0),
                                stop=(self_t == ntile - 1),
                            )
                        # normalize: rows 0..63 divided by row 64 (denominator)
                        den = den_pool.tile([1, 2, SQ], F32, tag="den")
                        nc.vector.reciprocal(den[:], opv[DK : DK + 1, :, :])
                        for hh in range(2):
                            denb = den_pool.tile([DK, SQ], F32, tag="denb")
                            nc.gpsimd.partition_broadcast(denb[:], den[0:1, hh, :])
                            nc.vector.tensor_tensor(
                                ot_sb[p][hh * DK : (hh + 1) * DK, js],
                                opv[0:DK, hh, :],
                                denb[:],
                                ALU.mult,
                            )

                    # ---- exchange this (half, j) ot chunk with the pair core
                    # (issued on the Pool queue right after the normalizes that
                    # produce it; overlaps the rest of the attention compute)
                    if all_reduce:
                        ag_in = dram_pool.tile(
                            [2, P, SQ], BF16, tag="agin", bufs=2 * SJ
                        )
                        for p in range(2):
                            nc.gpsimd.dma_start(ag_in[p], ot_sb[p][:, js])
                        ag_out = dram_pool.tile(
                            [2, 2, P, SQ], BF16, tag="agout", bufs=2 * SJ
                        )
                        nc.gpsimd.collective_compute(
                            "AllGather",
                            ALU.bypass,
                            replica_groups=groups,
                            ins=[ag_in.opt()],
                            outs=[ag_out.opt()],
                        )
                        ag_outs[(half, j)] = ag_out

                po = po_pool.tile([P, 4, SEQ], BF16, tag=f"po{half}")
                po_tiles[half] = po
                if not all_reduce:
                    for p in range(2):
                        nc.vector.tensor_copy(po[:, p, :], ot_sb[p][:])
                        nc.vector.tensor_copy(po[:, 2 + p, :], ot_sb[p][:])

            # ---------------- output projection (this core's 512 dims) ----------------
            # bf16 rhs allows 1024-wide moving dim: two SQ chunks per matmul
            _mark(nc, "outproj")
            for j in range(SJ):
                js = slice(j * SQ, (j + 1) * SQ)
                if all_reduce:
                    for half in range(2):
                        nc.sync.dma_start(
                            po_tiles[half][:, :, js],
                            ag_outs[(half, j)].rearrange("r c p s -> p (r c) s"),
                        )
            for j in range(SJ):
                js = slice(j * SQ, (j + 1) * SQ)
                for dc in range(OC):
                    ps = psA.tile([P, SQ], F32, tag=("mmE", "mmO")[dc % 2])
                    for ic in range(KC):
                        nc.tensor.matmul(
                            ps[:],
                            lhsT=(wo_sb[:, ic, dc * P : (dc + 1) * P]),
                            rhs=(po_tiles[ic // 4][:, ic % 4, js]),
                            start=(ic == 0),
                            stop=(ic == KC - 1),
                        )
                    ob = ob_pool.tile([P, SQ], F32, tag="ob")
                    nc.scalar.copy(ob[:], ps[:])
                    nc.gpsimd.dma_start(outp[dc * P : (dc + 1) * P, js], ob[:])

    nc.finalize()
    return nc


def make_perms():
    """perm (q/k rows): per half, E-block then O-block across the half's 4
    heads -- so the per-half projection psums are [4 heads x 32] E and O.
    perm2 (v/wo rows): per head, [even dims | odd dims].
    Both are local to a core's 512 rows (caller adds the head-group offset)."""
    perm = []
    for half in range(2):
        for par in range(2):  # 0=E, 1=O
            for h in range(4 * half, 4 * half + 4):
                for i in range(32):
                    perm.append(h * DK + 2 * i + par)
    perm2 = []
    for h in range(NH):
        for par in range(2):
            for i in range(32):
                perm2.append(h * DK + 2 * i + par)
    return np.array(perm), np.array(perm2)


def make_wot_order():
    """Global wo column order matching the gathered po layout:
    [half][rank][pair][head-in-pair][perm2-within-head]."""
    order = []
    for half in range(2):
        for rank in range(2):
            for pair in range(2):
                for hh in range(2):
                    g = rank * NH + half * 4 + pair * 2 + hh
                    for par in range(2):
                        for i in range(32):
                            order.append(g * DK + 2 * i + par)
    return np.array(order)


def make_tables(token_positions, SEQ):
    pos = np.asarray(token_positions).astype(np.float32)
    inv_freq = (1.0 / (THETA ** (np.arange(0, DK, 2, dtype=np.float32) / DK))).astype(
        np.float32
    )
    freqs = pos[:, None] * inv_freq[None, :]  # [S, 32]
    cosT = np.cos(freqs).T.astype(np.float32)  # [32, S]
    sinT = np.sin(freqs).T.astype(np.float32)
    return (
        np.ascontiguousarray(np.tile(cosT, (4, 1))),
        np.ascontiguousarray(np.tile(sinT, (4, 1))),
    )


def shard_inputs(x, token_positions, wq, wk, wv, wo):
    """Build the 8 per-core input maps."""
    perm, perm2 = make_perms()
    worder = make_wot_order()
    cos4, sin4 = make_tables(token_positions, x.shape[1])
    in_maps = []
    for c in range(N_CORES):
        b, hg = c // 2, c % 2
        rows = hg * DLOC
        gperm = perm + rows
        gperm2 = perm2 + rows
        wot = wo[hg * DLOC : (hg + 1) * DLOC, :][:, worder].T  # [1024, 512]
        in_maps.append(
            {
                "xt": np.ascontiguousarray(x[b].T),
                "wqt": np.ascontiguousarray(wq[gperm, :].T),
                "wkt": np.ascontiguousarray(wk[gperm, :].T),
                "wvt": np.ascontiguousarray(wv[gperm2, :].T),
                "wot": np.ascontiguousarray(wot).astype(ml_dtypes.bfloat16),
                "cos4": cos4,
                "sin4": sin4,
            }
        )
    return in_maps


_NC_CACHE = {}


def kernel(x, token_positions, wq, wk, wv, wo, trace=False):
    x = np.asarray(x, dtype=np.float32)
    wq = np.asarray(wq, dtype=np.float32)
    wk = np.asarray(wk, dtype=np.float32)
    wv = np.asarray(wv, dtype=np.float32)
    wo = np.asarray(wo, dtype=np.float32)

    key = "full"
    if key not in _NC_CACHE:
        _NC_CACHE[key] = build_attention_program(
            DIN=D,
            DOUT=D,
            SEQ=S,
            all_reduce=True,
            groups=[[0, 1], [2, 3], [4, 5], [6, 7]],
        )
    nc = _NC_CACHE[key]

    in_maps = shard_inputs(x, token_positions, wq, wk, wv, wo)
    res = run_bass_kernel_spmd(nc, in_maps, list(range(N_CORES)), trace=trace)
    out = np.empty((B, S, D), dtype=np.float32)
    for b in range(B):
        outT = np.concatenate(
            [res.results[2 * b]["outp"], res.results[2 * b + 1]["outp"]], axis=0
        )  # [D, S]
        out[b] = outT.T
    if trace:
        return out, res
    return out
